# revision 30
# baseline (speedup 1.0000x reference)
"""AttnDecoderRNN step on 8 Trainium2 NeuronCores (Bass/Tile, SPMD).

Sharding strategy (tensor-parallel over output dims, vocab-sharded big matvec):
  - Embedding lookup is pure data movement: done host-side (one row of emb).
  - GRU gates: W_ih/W_hh row-sharded (each core owns a 128-slice of H for all
    three gates) -> each core computes h_new for its slice. No comm.
  - Attention: attn_W column-sharded against the local h_new slice, fused with
    encoder_outputs @ q so a single AllReduce combines scores [400], the
    ws.h_new dot partial, full h_new (mask trick), and the combine-FF h-part
    partials; the FF a-part uses replicated weights, so no second AllReduce.
  - out projection [V,H] row-sharded 6250 rows/core (padded 6272), weights
    pre-transposed + bf16 on host; PE matvec with v on partitions so the
    softmax reduction is partition-parallel.
  - softmax over V: local sum of exp, AllGather of 8 scalars, log-sum-exp
    correction applied locally. (No max subtraction needed: logits are O(1)
    for this model scale; exp is safely inside fp32 range.)
Outputs: each core writes its vocab shard; core 0's h_new / attn_weights /
atten_p are used. Host gathers + undoes the column-major layout.
"""
import sys

sys.path.insert(0, "/opt/trn_rl_repo")

import numpy as np
import ml_dtypes

import concourse.bass as bass
import concourse.mybir as mybir
import concourse.tile as tile
from concourse.tile_rust import add_dep_helper
from concourse.vector_clock import ScopedClock
from concourse import bass_utils
from concourse.bass_utils import run_bass_kernel_spmd

# ---------------------------------------------------------------- patches ---
# This walrus build rejects >1 sync wait on a TPB_CTRL (Drain) instruction;
# TileContext's tail drain accumulates every outstanding sem wait onto it.
# Split the waits onto single-wait nops emitted just before the drain.


def _patched_drain_and_barrier(self, tick_clock, wait_clock):
    nc = self.nc
    carrier = nc.sync.nop(nofuse=True)
    wait_clock.add_sem_waits(carrier.ins, ScopedClock({None: tick_clock.global_clock}))
    si = carrier.ins.sync_info
    waits = list(si.on_wait) if si and si.on_wait else []
    if len(waits) > 1:
        carrier.ins.sync_info = mybir.SyncInfo(
            on_wait=[waits[0]], on_update=si.on_update
        )
        for w in waits[1:]:
            extra = nc.sync.nop(nofuse=True)
            esi = extra.ins.sync_info
            extra.ins.sync_info = mybir.SyncInfo(
                on_wait=[w], on_update=esi.on_update if esi else []
            )
    nc.sync.drain()
    nc.all_engine_barrier()
    popped = nc._tile_sem_poison_stack.pop()
    assert popped is self._sem_poison
    nc.clear_and_free_semaphores(list(self.sems.allocated().values()))
    nc.all_engine_barrier()


tile.TileContext._drain_and_barrier = _patched_drain_and_barrier

# Artifact upload needs a fish bucket; not available (and not needed) here.
bass_utils.upload_artifacts = lambda tmpdir: tmpdir



# This container's antenv lacks axon_hooks; provide the NTFF profile hook via
# ctypes into libaxon_pjrt.so (same shim trn_agent_boot would install).
def _install_ntff_hook_shim():
    import types
    import contextlib
    import ctypes

    if "antenv.axon_hooks" in sys.modules:
        return
    hook = None
    try:
        lib = ctypes.CDLL("/opt/axon/libaxon_pjrt.so")
        if hasattr(lib, "axon_start_nrt_profile"):
            lib.axon_start_nrt_profile.argtypes = [
                ctypes.POINTER(ctypes.c_int64),
                ctypes.c_size_t,
            ]
            lib.axon_start_nrt_profile.restype = ctypes.c_int64
            lib.axon_stop_nrt_profile.argtypes = [ctypes.c_char_p]
            lib.axon_stop_nrt_profile.restype = ctypes.c_int64

            @contextlib.contextmanager
            def _hook(output_dir, device_ids):
                import jax

                jax.devices()
                if device_ids:
                    ids = (ctypes.c_int64 * len(device_ids))(*device_ids)
                    rc = lib.axon_start_nrt_profile(ids, len(device_ids))
                else:
                    rc = lib.axon_start_nrt_profile(None, 0)
                if rc != 0:
                    raise RuntimeError(f"axon_start_nrt_profile rc={rc}")
                try:
                    yield
                finally:
                    n = lib.axon_stop_nrt_profile(str(output_dir).encode())
                    print(f"ntff profile: {n} file(s) -> {output_dir}",
                          file=sys.stderr)

            hook = _hook
    except OSError:
        pass
    mod = types.ModuleType("antenv.axon_hooks")
    mod.get_axon_ntff_profile_hook = lambda: hook
    mod.set_axon_ntff_profile_hook = lambda h: None
    sys.modules["antenv.axon_hooks"] = mod
    import antenv

    antenv.axon_hooks = mod


_install_ntff_hook_shim()

# ------------------------------------------------------------- constants ---
NC_N = 8
H = 1024
V = 50000
E = 602
S = 400
P = 128
VR = V // NC_N        # 6250 real vocab rows per core
MT = 49               # vocab m-tiles per core
VP = MT * P           # 6272 padded vocab rows per core
SP = 512              # padded S
ET_M = 4              # s-tiles (512/128)
EP_COLS = 640         # padded E (5*128)
APT = 5               # atten m-tiles

# packed-input free-dim offsets (bf16 elements per partition), 4 packs in
# arrival-priority order: p1a (GRU input weights) -> p1b (GRU hidden weights)
# -> p2a (attention score weights) -> p2b (everything else)
OFF_WIH = 0
OFF_X = OFF_WIH + 16 * 384
OFF_HV = OFF_X + 16
NB1A = OFF_HV + 16
OFF_WHH = 0
NB1B = 8 * 384
OFF_ATTN = 0
OFF_ET = OFF_ATTN + 1024
NB2A = OFF_ET + 8 * 512
OFF_EP = 0
OFF_CH = OFF_EP + 4 * 1024
OFF_CA = OFF_CH + 1024
OFF_PG = OFF_CA + 8 * 1024
OFF_WH = OFF_PG + 4 * 640
OFF_WS = OFF_WH + 16
OFF_WX = OFF_WS + 16
OFF_EMB = OFF_WX + 16
NB2B = OFF_EMB + 16
# f32 pack offsets
OFF_OUTB = 0
OFF_B01 = 56
OFF_BIHN = 64
OFF_BHHN = 72
OFF_CB = 80
OFF_HCOL = 88
OFF_CMASK = 96
OFF_SMASK = 104
OFF_CONSTS = 112
OFF_IDENT = 120
NF = 248

F32 = mybir.dt.float32
BF16 = mybir.dt.bfloat16
NPBF = ml_dtypes.bfloat16

LAST_RESULT = None    # BassKernelResults of the most recent run (for test.py)
TRACE = False         # set True (e.g. by test.py) to profile
DEBUG = False         # add per-stage debug outputs

_NC_CACHE = None



def _split_multi_waits(nc):
    """This walrus build accepts a single sync wait per instruction; hoist
    extra waits onto same-engine nops placed just before the instruction."""
    for f in nc.m.functions:
        for bb in f.blocks:
            out = []
            for ins in bb.instructions:
                si = ins.sync_info
                waits = list(si.on_wait) if si and si.on_wait else []
                if len(waits) > 1:
                    for w in waits[:-1]:
                        nop = mybir.InstNoOp(
                            name=nc.get_next_instruction_name(),
                            engine=ins.engine,
                            ins=[],
                            outs=[],
                            sync_info=mybir.SyncInfo(on_wait=[w], on_update=[]),
                        )
                        out.append(nop)
                    ins.sync_info = mybir.SyncInfo(
                        on_wait=[waits[-1]], on_update=si.on_update
                    )
                out.append(ins)
            bb.instructions = out


# ------------------------------------------------------------ device code ---
def _build_nc():
    nc = bass.Bass()

    def di(name, shape, dt=BF16):
        return nc.dram_tensor(name, shape, dt, kind="ExternalInput")

    # per-core inputs: one packed bf16 tensor, one packed f32 tensor, big W
    wo_t = di("wo_t", [H, VP], mybir.dt.float8e3)
    pack1a = di("pack1a", [P, NB1A])
    pack1b = di("pack1b", [P, NB1B])
    pack2a = di("pack2a", [P, NB2A])
    pack2b = di("pack2b", [P, NB2B])
    pack_f32 = di("pack_f32", [P, NF], F32)

    vocab_out = nc.dram_tensor("vocab_out", [P, MT], F32, kind="ExternalOutput")
    hnew_out = nc.dram_tensor("hnew_out", [P, 8], F32, kind="ExternalOutput")
    attnw_out = nc.dram_tensor("attnw_out", [P, 4], F32, kind="ExternalOutput")
    atten_out = nc.dram_tensor("atten_out", [P, APT], F32, kind="ExternalOutput")
    if DEBUG:
        dbg = {
            name: nc.dram_tensor(name, shape, F32, kind="ExternalOutput")
            for name, shape in [
                ("dbg_gi", [P, 3]), ("dbg_gh", [P, 3]), ("dbg_hnewl", [P, 1]),
                ("dbg_qp", [P, 8]), ("dbg_scl", [P, 4]), ("dbg_ar1", [P, 16]),
                ("dbg_aa", [P, 8]), ("dbg_ff", [P, 8]), ("dbg_pgen", [1, 1]),
            ]
        }

    with tile.TileContext(nc) as tc:
        with (
            tc.tile_pool(name="wp", bufs=1) as wp,
            tc.tile_pool(name="sp", bufs=1) as spool,
            tc.tile_pool(name="pp", bufs=1, space="PSUM") as pp,
            tc.tile_pool(name="bigp", bufs=1, space="PSUM") as bigp,
            tc.tile_pool(name="dram", bufs=1, space="DRAM") as dp,
        ):
            # ---- SBUF loads: 2 packed DMAs + 8 big-W chunk DMAs ----
            # (each dma_start costs ~1.1us of serial issue time on its queue,
            # so everything small rides in two packed transfers)
            pf32 = wp.tile([P, NF], F32, tag="pf32", name="pf32")
            nc.sync.dma_start(out=pf32[:], in_=pack_f32[:])
            pk1a = wp.tile([P, NB1A], BF16, tag="pk1a", name="pk1a")
            d1a = nc.sync.dma_start(out=pk1a[:], in_=pack1a[:])
            pk1b = wp.tile([P, NB1B], BF16, tag="pk1b", name="pk1b")
            d1b = nc.sync.dma_start(out=pk1b[:], in_=pack1b[:])
            pk2a = wp.tile([P, NB2A], BF16, tag="pk2a", name="pk2a")
            d2a = nc.sync.dma_start(out=pk2a[:], in_=pack2a[:])
            pk2b = wp.tile([P, NB2B], BF16, tag="pk2b", name="pk2b")
            d2b = nc.sync.dma_start(out=pk2b[:], in_=pack2b[:])
            add_dep_helper(d1b.ins, d1a.ins, reason="arrival priority")
            add_dep_helper(d2a.ins, d1b.ins, reason="arrival priority")
            add_dep_helper(d2b.ins, d2a.ins, reason="arrival priority")
            wo_sb = wp.tile([P, 8, VP], mybir.dt.float8e3, tag="wo", name="wo")
            for k in range(8):
                d_wo = nc.scalar.dma_start(
                    out=wo_sb[:, k, :], in_=wo_t[P * k : P * (k + 1), :]
                )
                add_dep_helper(d_wo.ins, d2a.ins, reason="packs get HBM first")

            wih = lambda k, m: pk1a[:, OFF_WIH + k * 384 + m * P :][:, :P]
            x_k = lambda k: pk1a[:, OFF_X + k : OFF_X + k + 1]
            h_k = lambda k: pk1a[:, OFF_HV + k : OFF_HV + k + 1]
            whh = lambda k, m: pk1b[:, OFF_WHH + k * 384 + m * P :][:, :P]
            attn_m = lambda m: pk2a[:, OFF_ATTN + m * P : OFF_ATTN + (m + 1) * P]
            et_km = lambda k, m: pk2a[:, OFF_ET + k * SP + m * P :][:, :P]
            ep_km = lambda k, m: pk2b[:, OFF_EP + k * H + m * P :][:, :P]
            ch_m = lambda m: pk2b[:, OFF_CH + m * P : OFF_CH + (m + 1) * P]
            ca_km = lambda k, m: pk2b[:, OFF_CA + k * H + m * P :][:, :P]
            pg_km = lambda k, m: pk2b[:, OFF_PG + k * EP_COLS + m * P :][:, :P]
            wh_k = lambda k: pk2b[:, OFF_WH + k : OFF_WH + k + 1]
            ws_sb = pk2b[:, OFF_WS : OFF_WS + 1]
            wx_k = lambda k: pk2b[:, OFF_WX + k : OFF_WX + k + 1]
            emb_k = lambda k: pk2b[:, OFF_EMB + k : OFF_EMB + k + 1]

            outb_sb = pf32[:, OFF_OUTB : OFF_OUTB + MT]
            b01_sb = pf32[:, OFF_B01 : OFF_B01 + 2]
            bihn_sb = pf32[:, OFF_BIHN : OFF_BIHN + 1]
            bhhn_sb = pf32[:, OFF_BHHN : OFF_BHHN + 1]
            cb_sb = pf32[:, OFF_CB : OFF_CB + 8]
            hcol_sb = pf32[:, OFF_HCOL : OFF_HCOL + 1]
            cmask_sb = pf32[:, OFF_CMASK : OFF_CMASK + 8]
            smask_sb = pf32[:, OFF_SMASK : OFF_SMASK + 4]
            consts_sb = pf32[0:1, OFF_CONSTS : OFF_CONSTS + 8]
            id_sb = pf32[:, OFF_IDENT : OFF_IDENT + P]

            onesc = spool.tile([P, 1], F32, tag="onesc")
            nc.vector.memset(onesc[:], 1.0)
            onesr = spool.tile([1, P], F32, tag="onesr")
            nc.vector.memset(onesr[:], 1.0)

            def psum(shape, tag="tiny", bufs=2, name="ps"):
                return pp.tile(list(shape), F32, tag=tag, bufs=bufs, name=name)

            def part_sum(vec_sb, k=P):
                """sum over partitions of [k,1] f32 -> [1,1] psum"""
                out = psum([1, 1])
                nc.tensor.matmul(out[:], onesc[:k, :], vec_sb, start=True, stop=True)
                return out

            def bcast(scalar_sb):
                """[1,1] sbuf f32 -> [128,1] sbuf f32"""
                pb = psum([P, 1])
                nc.tensor.matmul(pb[:], onesr[:], scalar_sb, start=True, stop=True)
                sb = spool.tile([P, 1], F32, tag="bc", name="bc")
                nc.vector.tensor_copy(sb[:], pb[:])
                return sb

            def to_sb(ps, shape, dt=F32, tag="cp"):
                sb = spool.tile(list(shape), dt, tag=tag, name=tag)
                nc.vector.tensor_copy(sb[:], ps)
                return sb

            # ---- stage 1: GRU slice (no comm) ----
            gi = psum([P, 3], tag="gates", name="gi")
            for k in range(16):
                for m in range(3):
                    nc.tensor.matmul(
                        gi[:, m : m + 1],
                        wih(k, m),
                        x_k(k),
                        start=(k == 0 and m == 0),
                        stop=(k == 15 and m == 2),
                    )
            gh = psum([P, 3], tag="gates", name="gh")
            for k in range(8):
                for m in range(3):
                    nc.tensor.matmul(
                        gh[:, m : m + 1],
                        whh(k, m),
                        h_k(k),
                        start=(k == 0 and m == 0),
                        stop=(k == 7 and m == 2),
                    )
            gi_sb = to_sb(gi[:], [P, 3], F32, tag="gisb")
            t01 = spool.tile([P, 2], F32, tag="t01")
            nc.vector.tensor_add(t01[:], gi_sb[:, 0:2], gh[:, 0:2])
            nc.vector.tensor_add(t01[:], t01[:], b01_sb)
            rz = spool.tile([P, 2], F32, tag="rz")
            nc.scalar.activation(rz[:], t01[:], mybir.ActivationFunctionType.Sigmoid)

            ghn = spool.tile([P, 1], F32, tag="ghn")
            nc.vector.tensor_add(ghn[:], gh[:, 2:3], bhhn_sb)
            tmp1 = spool.tile([P, 1], F32, tag="tmp1")
            nc.vector.tensor_mul(tmp1[:], rz[:, 0:1], ghn[:])
            npre = spool.tile([P, 1], F32, tag="npre")
            nc.vector.tensor_add(npre[:], gi_sb[:, 2:3], bihn_sb)
            nc.vector.tensor_add(npre[:], npre[:], tmp1[:])
            n_sb = spool.tile([P, 1], F32, tag="n")
            nc.scalar.activation(n_sb[:], npre[:], mybir.ActivationFunctionType.Tanh)
            warm = spool.tile([1, 1], F32, tag="warm")
            nc.scalar.activation(warm[:], consts_sb[0:1, 0:1],
                                 mybir.ActivationFunctionType.Exp)
            nc.scalar.activation(warm[:], consts_sb[0:1, 0:1],
                                 mybir.ActivationFunctionType.Ln)
            # h_new = n + z*(h - n)
            d_sb = spool.tile([P, 1], F32, tag="d")
            nc.vector.tensor_sub(d_sb[:], hcol_sb, n_sb[:])
            zt = spool.tile([P, 1], F32, tag="zt")
            nc.vector.tensor_mul(zt[:], rz[:, 1:2], d_sb[:])
            hnew = spool.tile([P, 1], F32, tag="hnew")
            nc.vector.tensor_add(hnew[:], n_sb[:], zt[:])
            hnew_bf = to_sb(hnew[:], [P, 1], BF16, tag="hnewbf")
            if DEBUG:
                nc.sync.dma_start(out=dbg["dbg_hnewl"][:], in_=hnew[:])
                nc.sync.dma_start(out=dbg["dbg_gi"][:], in_=gi_sb[:])
                gh_dbg = to_sb(gh[:], [P, 3], F32, tag="ghdbg")
                nc.sync.dma_start(out=dbg["dbg_gh"][:], in_=gh_dbg[:])

            # ---- stage 2: partial attention scores ----
            qp = psum([P, 8], tag="vec8", name="qp")
            for m in range(8):
                nc.tensor.matmul(
                    qp[:, m : m + 1],
                    attn_m(m),
                    hnew_bf[:],
                    start=(m == 0),
                    stop=(m == 7),
                )
            qp_bf = to_sb(qp[:], [P, 8], BF16, tag="qpbf")
            sc = psum([P, 4], tag="vec8", name="sc")
            for k in range(8):
                for m in range(4):
                    nc.tensor.matmul(
                        sc[:, m : m + 1],
                        et_km(k, m),
                        qp_bf[:, k : k + 1],
                        start=(k == 0 and m == 0),
                        stop=(k == 7 and m == 3),
                    )
            if DEBUG:
                qp_dbg = to_sb(qp[:], [P, 8], F32, tag="qpdbg")
                nc.sync.dma_start(out=dbg["dbg_qp"][:], in_=qp_dbg[:])
                sc_dbg = to_sb(sc[:], [P, 4], F32, tag="scdbg")
                nc.sync.dma_start(out=dbg["dbg_scl"][:], in_=sc_dbg[:])
            wsp = psum([1, 1], name="wsp")
            nc.tensor.matmul(wsp[:], ws_sb, hnew_bf[:], start=True, stop=True)
            # h-part of the combine FF, against the local h_new slice; rides
            # AllReduce #1 so the a-part (replicated) is all that remains after
            pfh = psum([P, 8], tag="vec8", name="pfh")
            for m in range(8):
                nc.tensor.matmul(
                    pfh[:, m : m + 1],
                    ch_m(m),
                    hnew_bf[:],
                    start=(m == 0),
                    stop=(m == 7),
                )

            # ---- AllReduce #1: scores + ws + h_new + ff-h-part ----
            ar1 = spool.tile([P, 24], F32, tag="ar1")
            nc.vector.memset(ar1[:], 0.0)
            nc.vector.tensor_copy(ar1[:, 0:4], sc[:])
            nc.vector.tensor_copy(ar1[0:1, 4:5], wsp[:])
            nc.vector.tensor_scalar_mul(ar1[:, 5:13], cmask_sb, hnew[:])
            nc.vector.tensor_copy(ar1[:, 16:24], pfh[:])
            ar1_in = dp.tile([P, 24], F32)
            ar1_out = dp.tile([P, 24], F32)
            nc.sync.dma_start(out=ar1_in[:], in_=ar1[:])
            nc.gpsimd.collective_compute(
                "AllReduce",
                mybir.AluOpType.add,
                replica_groups=[list(range(NC_N))],
                ins=[ar1_in.opt()],
                outs=[ar1_out.opt()],
            )
            # PE keep-warm during the AllReduce wait (dependency-free)
            warm_ps = psum([P, 1], tag="warmps", bufs=1, name="warmps")
            for w in range(60):
                nc.tensor.matmul(
                    warm_ps[:],
                    ch_m(w % 8),
                    hnew_bf[:],
                    start=(w == 0),
                    stop=(w == 59),
                )
            ag1 = spool.tile([P, 24], F32, tag="ag1")
            nc.sync.dma_start(out=ag1[:], in_=ar1_out[:])
            nc.sync.dma_start(out=hnew_out[:], in_=ag1[:, 5:13])
            if DEBUG:
                nc.sync.dma_start(out=dbg["dbg_ar1"][:], in_=ag1[:])

            # ---- stage 3: softmax(scores), attn_applied, p_gen (replicated) ----
            # scores span +-45 for this model scale -> exp safe in fp32
            # without max subtraction; softmax normalization is deferred so
            # the attn-applied matmuls start straight off the exp.
            scores = spool.tile([P, 4], F32, tag="scores")
            nc.vector.tensor_add(scores[:], ag1[:, 0:4], smask_sb)
            expsc = spool.tile([P, 4], F32, tag="expsc")
            rsum = spool.tile([P, 1], F32, tag="rsum")
            nc.scalar.activation(
                expsc[:], scores[:], mybir.ActivationFunctionType.Exp,
                accum_out=rsum[:],
            )
            aw_bf = to_sb(expsc[:], [P, 4], BF16, tag="awbf")  # unnormalized
            stot = to_sb(part_sum(rsum[:])[:], [1, 1], tag="stot")
            rinv = spool.tile([1, 1], F32, tag="rinv")
            nc.vector.reciprocal(rinv[:], stot[:])
            rinv_b = bcast(rinv[:])
            aw = spool.tile([P, 4], F32, tag="aw")
            nc.vector.tensor_scalar_mul(aw[:], expsc[:], rinv_b[:])
            nc.sync.dma_start(out=attnw_out[:], in_=aw[:])
            awn_bf = to_sb(aw[:], [P, 4], BF16, tag="awnbf")

            # attn_applied = attn_weights @ E  (full, replicated)
            aa = psum([P, 8], tag="vec8", name="aa")
            for k in range(ET_M):
                for m in range(8):
                    nc.tensor.matmul(
                        aa[:, m : m + 1],
                        ep_km(k, m),
                        aw_bf[:, k : k + 1],
                        start=(k == 0 and m == 0),
                        stop=(k == ET_M - 1 and m == 7),
                    )
            aa_bf = spool.tile([P, 8], BF16, tag="aabf")
            nc.vector.tensor_scalar_mul(aa_bf[:], aa[:], rinv_b[:])
            # a-part of the combine FF against full (replicated) weights
            pfa = psum([P, 8], tag="vec8", name="pfa")
            for k in range(8):
                for m in range(8):
                    nc.tensor.matmul(
                        pfa[:, m : m + 1],
                        ca_km(k, m),
                        aa_bf[:, k : k + 1],
                        start=(k == 0 and m == 0),
                        stop=(k == 7 and m == 7),
                    )
            pff = spool.tile([P, 8], F32, tag="pff")
            nc.vector.tensor_add(pff[:], ag1[:, 16:24], pfa[:])
            nc.vector.tensor_add(pff[:], pff[:], cb_sb)
            ff_bf = spool.tile([P, 8], BF16, tag="ffbf")
            nc.scalar.activation(ff_bf[:], pff[:], mybir.ActivationFunctionType.Relu)
            if DEBUG:
                ff = spool.tile([P, 8], F32, tag="ff")
                nc.scalar.activation(ff[:], pff[:],
                                     mybir.ActivationFunctionType.Relu)
                nc.sync.dma_start(out=dbg["dbg_ff"][:], in_=ff[:])

            # ---- big matvec: logits shard [128, 49] ----
            big = bigp.tile([P, MT], F32, tag="big")
            for k in range(8):
                for j in range(MT):
                    nc.tensor.matmul(
                        big[:, j : j + 1],
                        wo_sb[:, k, j * P : (j + 1) * P],
                        ff_bf[:, k : k + 1],
                        start=(k == 0 and j == 0),
                        stop=(k == 7 and j == MT - 1),
                    )
            logits = spool.tile([P, MT], F32, tag="logits")
            nc.vector.scalar_tensor_tensor(
                logits[:], big[:], 1.0 / 64.0, outb_sb,
                op0=mybir.AluOpType.mult, op1=mybir.AluOpType.add,
            )
            expv = spool.tile([P, MT], F32, tag="expv")
            esum = spool.tile([P, 1], F32, tag="esum")
            nc.scalar.activation(
                expv[:], logits[:], mybir.ActivationFunctionType.Exp,
                accum_out=esum[:],
            )
            se = to_sb(part_sum(esum[:])[:], [1, 1], tag="se")

            # ---- AllGather #3: per-core sumexp ----
            ag3 = spool.tile([1, 8], F32, tag="ag3")
            nc.vector.memset(ag3[:], 0.0)
            nc.vector.tensor_copy(ag3[0:1, 0:1], se[:])
            ag3_in = dp.tile([1, 8], F32)
            # AG concat is on the partition axis, but DRAM is linear: a
            # [1, 64] view of the same bytes gives the 8 rank rows flat.
            ag3_out = dp.tile([1, 64], F32)
            nc.sync.dma_start(out=ag3_in[:], in_=ag3[:])
            nc.gpsimd.collective_compute(
                "AllGather",
                mybir.AluOpType.bypass,
                replica_groups=[list(range(NC_N))],
                ins=[ag3_in.opt()],
                outs=[ag3_out.opt()],
            )
            # p_gen + atten_p: independent of the vocab chain; scheduled here
            # so they fill the AllGather wait window
            pgp = psum([1, 1], name="pgp")
            for k in range(8):
                nc.tensor.matmul(
                    pgp[:], wh_k(k), aa_bf[:, k : k + 1],
                    start=(k == 0), stop=False,
                )
            for k in range(8):
                nc.tensor.matmul(
                    pgp[:], wx_k(k), emb_k(k),
                    start=False, stop=(k == 7),
                )
            p1 = spool.tile([1, 1], F32, tag="p1")
            nc.vector.tensor_add(p1[:], pgp[:], ag1[0:1, 4:5])
            pgen = spool.tile([1, 1], F32, tag="pgen")
            nc.scalar.activation(
                pgen[:], p1[:], mybir.ActivationFunctionType.Sigmoid,
                bias=consts_sb[0:1, 1:2],
            )
            ln_pg = spool.tile([1, 1], F32, tag="lnpg")
            nc.scalar.activation(ln_pg[:], pgen[:], mybir.ActivationFunctionType.Ln)
            om = spool.tile([1, 1], F32, tag="om")
            nc.vector.tensor_sub(om[:], consts_sb[0:1, 0:1], pgen[:])
            ln_om = spool.tile([1, 1], F32, tag="lnom")
            nc.scalar.activation(ln_om[:], om[:], mybir.ActivationFunctionType.Ln)
            if DEBUG:
                nc.sync.dma_start(out=dbg["dbg_pgen"][:], in_=pgen[:])

            app = psum([P, APT], tag="vec8", name="app")
            for k in range(ET_M):
                for m in range(APT):
                    nc.tensor.matmul(
                        app[:, m : m + 1],
                        pg_km(k, m),
                        awn_bf[:, k : k + 1],
                        start=(k == 0 and m == 0),
                        stop=(k == ET_M - 1 and m == APT - 1),
                    )
            ln_ap = spool.tile([P, APT], F32, tag="lnap")
            nc.scalar.activation(ln_ap[:], app[:], mybir.ActivationFunctionType.Ln)
            lnom_b = bcast(ln_om[:])
            apf = spool.tile([P, APT], F32, tag="apf")
            nc.vector.tensor_scalar_add(apf[:], ln_ap[:], lnom_b[:])
            nc.sync.dma_start(out=atten_out[:], in_=apf[:])

            agd = spool.tile([1, 64], F32, tag="agd")
            nc.sync.dma_start(out=agd[:], in_=ag3_out[:])
            tot = spool.tile([1, 1], F32, tag="tot")
            nc.vector.reduce_sum(out=tot[:], in_=agd[:], axis=mybir.AxisListType.X)
            lnz = spool.tile([1, 1], F32, tag="lnz")
            nc.scalar.activation(lnz[:], tot[:], mybir.ActivationFunctionType.Ln)
            corr = spool.tile([1, 1], F32, tag="corr")
            nc.vector.tensor_sub(corr[:], lnz[:], ln_pg[:])
            corr_b = bcast(corr[:])
            final = spool.tile([P, MT], F32, tag="final")
            nc.vector.tensor_scalar_sub(final[:], logits[:], corr_b[:])
            nc.sync.dma_start(out=vocab_out[:], in_=final[:])

    _split_multi_waits(nc)
    return nc


# -------------------------------------------------------------- host side ---
def _colmajor(v, ncol):
    return np.ascontiguousarray(v.reshape(ncol, P).T)


def _prep_inputs(inputs):
    f32 = np.float32
    idx = int(np.asarray(inputs["input_idx"]).ravel()[0])
    emb = np.asarray(inputs["emb"], f32)
    embedded = emb[idx]
    trigger = np.asarray(inputs["trigger"], f32)
    x = np.concatenate([embedded, trigger])
    h = np.asarray(inputs["hidden"], f32)[0, 0]
    enc = np.asarray(inputs["encoder_outputs"], f32)
    pg_mat = np.asarray(inputs["pg_mat"], f32)
    attn_W = np.asarray(inputs["attn_W"], f32)
    comb_W = np.asarray(inputs["comb_W"], f32)
    comb_b = np.asarray(inputs["comb_b"], f32)
    W_ih = np.asarray(inputs["W_ih"], f32)
    W_hh = np.asarray(inputs["W_hh"], f32)
    b_ih = np.asarray(inputs["b_ih"], f32)
    b_hh = np.asarray(inputs["b_hh"], f32)
    out_W = np.asarray(inputs["out_W"], f32)
    out_b = np.asarray(inputs["out_b"], f32)
    wh_W = np.asarray(inputs["wh_W"], f32)[0]
    ws_W = np.asarray(inputs["ws_W"], f32)[0]
    wx_W = np.asarray(inputs["wx_W"], f32)[0]
    wx_b = np.asarray(inputs["wx_b"], f32)[0]

    et = np.zeros((H, SP), f32)
    et[:, :S] = enc.T
    ep = np.zeros((SP, H), f32)
    ep[:S] = enc
    pgp = np.zeros((SP, EP_COLS), f32)
    pgp[:S, :E] = pg_mat
    pgp[:S, E:] = 1.0
    sm_flat = np.zeros(SP, f32)
    sm_flat[S:] = -1e30

    def chunked(a, k, m):
        """[k*128, m] row-major -> [128, k*m]  ("(k p) m -> p (k m)")"""
        return a.reshape(k, P, m).transpose(1, 0, 2).reshape(P, k * m)

    def pad16(col):
        out = np.zeros((P, 16), f32)
        out[:, : col.shape[1]] = col
        return out

    # shared bf16 pack pieces (order must match OFF_* in the device code)
    et_pk = chunked(et, 8, SP)
    ep_pk = chunked(ep, ET_M, H)
    pg_pk = chunked(pgp, ET_M, EP_COLS)
    caT_pk = chunked(np.ascontiguousarray(comb_W[:, H:].T), 8, H)
    wh_pk = pad16(_colmajor(wh_W, 8))
    wx_pk = pad16(_colmajor(wx_W, 8))
    x_pk = pad16(_colmajor(x, 16))
    h_pk = pad16(_colmajor(h, 8))
    emb_pk = pad16(_colmajor(embedded, 8))

    ident = np.eye(P, dtype=f32)
    consts_col = np.zeros((P, 8), f32)
    consts_col[0, 0] = 1.0
    consts_col[0, 1] = wx_b
    smask_col = _colmajor(sm_flat, 4)
    cb_col = _colmajor(comb_b, 8)

    in_maps = []
    for c in range(NC_N):
        s = slice(P * c, P * (c + 1))
        rows = np.r_[P * c : P * (c + 1), H + P * c : H + P * (c + 1),
                     2 * H + P * c : 2 * H + P * (c + 1)]
        wsh = np.zeros((VP, H), f32)
        wsh[:VR] = out_W[VR * c : VR * (c + 1)]
        ob = np.full(VP, -40.0, f32)
        ob[:VR] = out_b[VR * c : VR * (c + 1)]
        cmask = np.zeros((P, 8), f32)
        cmask[:, c] = 1.0

        p1a = np.zeros((P, NB1A), f32)
        p1a[:, OFF_WIH : OFF_WIH + 16 * 384] = chunked(
            np.ascontiguousarray(W_ih[rows].T), 16, 384)
        p1a[:, OFF_X : OFF_X + 16] = x_pk
        p1a[:, OFF_HV : OFF_HV + 16] = h_pk

        p1b = np.zeros((P, NB1B), f32)
        p1b[:, OFF_WHH : OFF_WHH + 8 * 384] = chunked(
            np.ascontiguousarray(W_hh[rows].T), 8, 384)

        p2a = np.zeros((P, NB2A), f32)
        p2a[:, OFF_ATTN : OFF_ATTN + H] = attn_W[:, s].T
        p2a[:, OFF_ET : OFF_ET + 8 * SP] = et_pk

        p2b = np.zeros((P, NB2B), f32)
        p2b[:, OFF_EP : OFF_EP + ET_M * H] = ep_pk
        p2b[:, OFF_CH : OFF_CH + H] = comb_W[:, s].T
        p2b[:, OFF_CA : OFF_CA + 8 * H] = caT_pk
        p2b[:, OFF_PG : OFF_PG + ET_M * EP_COLS] = pg_pk
        p2b[:, OFF_WH : OFF_WH + 16] = wh_pk
        p2b[:, OFF_WS] = ws_W[s]
        p2b[:, OFF_WX : OFF_WX + 16] = wx_pk
        p2b[:, OFF_EMB : OFF_EMB + 16] = emb_pk

        pack_f32 = np.zeros((P, NF), f32)
        pack_f32[:, OFF_OUTB : OFF_OUTB + MT] = _colmajor(ob, MT)
        pack_f32[:, OFF_B01] = b_ih[s] + b_hh[s]
        pack_f32[:, OFF_B01 + 1] = (b_ih[H + P * c : H + P * (c + 1)]
                                    + b_hh[H + P * c : H + P * (c + 1)])
        pack_f32[:, OFF_BIHN] = b_ih[2 * H + P * c : 2 * H + P * (c + 1)]
        pack_f32[:, OFF_BHHN] = b_hh[2 * H + P * c : 2 * H + P * (c + 1)]
        pack_f32[:, OFF_CB : OFF_CB + 8] = cb_col
        pack_f32[:, OFF_HCOL] = h[s]
        pack_f32[:, OFF_CMASK : OFF_CMASK + 8] = cmask
        pack_f32[:, OFF_SMASK : OFF_SMASK + 4] = smask_col
        pack_f32[:, OFF_CONSTS : OFF_CONSTS + 8] = consts_col
        pack_f32[:, OFF_IDENT : OFF_IDENT + P] = ident

        m = {
            "wo_t": (np.ascontiguousarray(wsh.T) * 64.0).astype(ml_dtypes.float8_e3m4),
            "pack1a": p1a.astype(NPBF),
            "pack1b": p1b.astype(NPBF),
            "pack2a": p2a.astype(NPBF),
            "pack2b": p2b.astype(NPBF),
            "pack_f32": pack_f32,
        }
        in_maps.append(m)
    return in_maps


def kernel(**inputs):
    global _NC_CACHE, LAST_RESULT
    in_maps = _prep_inputs(inputs)
    if _NC_CACHE is None:
        _NC_CACHE = _build_nc()
    res = run_bass_kernel_spmd(
        _NC_CACHE, in_maps, list(range(NC_N)), trace=TRACE
    )
    LAST_RESULT = res

    vocab = np.concatenate(
        [res.results[c]["vocab_out"].T.reshape(-1)[:VR] for c in range(NC_N)]
    )
    atten = res.results[0]["atten_out"].T.reshape(-1)[:E]
    output = np.concatenate([vocab, atten])[None, :].astype(np.float32)
    h_new = res.results[0]["hnew_out"].T.reshape(-1)[None, None, :].astype(np.float32)
    attn_weights = (
        res.results[0]["attnw_out"].T.reshape(-1)[:S][None, :].astype(np.float32)
    )
    return output, h_new, attn_weights


# revision 31
# speedup vs baseline: 1.1514x; 1.1514x over previous
"""AttnDecoderRNN step on 8 Trainium2 NeuronCores (Bass/Tile, SPMD).

Sharding strategy (tensor-parallel over output dims, vocab-sharded big matvec):
  - Embedding lookup is pure data movement: done host-side (one row of emb).
  - GRU gates: W_ih/W_hh row-sharded (each core owns a 128-slice of H for all
    three gates) -> each core computes h_new for its slice. No comm.
  - Attention: attn_W column-sharded against the local h_new slice, fused with
    encoder_outputs @ q so a single AllReduce combines scores [400], the
    ws.h_new dot partial, full h_new (mask trick), and the combine-FF h-part
    partials; the FF a-part uses replicated weights, so no second AllReduce.
  - out projection [V,H] row-sharded 6250 rows/core (padded 6272), weights
    pre-transposed + bf16 on host; PE matvec with v on partitions so the
    softmax reduction is partition-parallel.
  - softmax over V: local sum of exp, AllGather of 8 scalars, log-sum-exp
    correction applied locally. (No max subtraction needed: logits are O(1)
    for this model scale; exp is safely inside fp32 range.)
Outputs: each core writes its vocab shard; core 0's h_new / attn_weights /
atten_p are used. Host gathers + undoes the column-major layout.
"""
import sys

sys.path.insert(0, "/opt/trn_rl_repo")

import numpy as np
import ml_dtypes

import concourse.bass as bass
import concourse.mybir as mybir
import concourse.tile as tile
from concourse.tile_rust import add_dep_helper
from concourse.vector_clock import ScopedClock
from concourse import bass_utils
from concourse.bass_utils import run_bass_kernel_spmd

# ---------------------------------------------------------------- patches ---
# This walrus build rejects >1 sync wait on a TPB_CTRL (Drain) instruction;
# TileContext's tail drain accumulates every outstanding sem wait onto it.
# Split the waits onto single-wait nops emitted just before the drain.


def _patched_drain_and_barrier(self, tick_clock, wait_clock):
    nc = self.nc
    carrier = nc.sync.nop(nofuse=True)
    wait_clock.add_sem_waits(carrier.ins, ScopedClock({None: tick_clock.global_clock}))
    si = carrier.ins.sync_info
    waits = list(si.on_wait) if si and si.on_wait else []
    if len(waits) > 1:
        carrier.ins.sync_info = mybir.SyncInfo(
            on_wait=[waits[0]], on_update=si.on_update
        )
        for w in waits[1:]:
            extra = nc.sync.nop(nofuse=True)
            esi = extra.ins.sync_info
            extra.ins.sync_info = mybir.SyncInfo(
                on_wait=[w], on_update=esi.on_update if esi else []
            )
    nc.sync.drain()
    nc.all_engine_barrier()
    popped = nc._tile_sem_poison_stack.pop()
    assert popped is self._sem_poison
    nc.clear_and_free_semaphores(list(self.sems.allocated().values()))
    nc.all_engine_barrier()


tile.TileContext._drain_and_barrier = _patched_drain_and_barrier

# Artifact upload needs a fish bucket; not available (and not needed) here.
bass_utils.upload_artifacts = lambda tmpdir: tmpdir



# This container's antenv lacks axon_hooks; provide the NTFF profile hook via
# ctypes into libaxon_pjrt.so (same shim trn_agent_boot would install).
def _install_ntff_hook_shim():
    import types
    import contextlib
    import ctypes

    if "antenv.axon_hooks" in sys.modules:
        return
    hook = None
    try:
        lib = ctypes.CDLL("/opt/axon/libaxon_pjrt.so")
        if hasattr(lib, "axon_start_nrt_profile"):
            lib.axon_start_nrt_profile.argtypes = [
                ctypes.POINTER(ctypes.c_int64),
                ctypes.c_size_t,
            ]
            lib.axon_start_nrt_profile.restype = ctypes.c_int64
            lib.axon_stop_nrt_profile.argtypes = [ctypes.c_char_p]
            lib.axon_stop_nrt_profile.restype = ctypes.c_int64

            @contextlib.contextmanager
            def _hook(output_dir, device_ids):
                import jax

                jax.devices()
                if device_ids:
                    ids = (ctypes.c_int64 * len(device_ids))(*device_ids)
                    rc = lib.axon_start_nrt_profile(ids, len(device_ids))
                else:
                    rc = lib.axon_start_nrt_profile(None, 0)
                if rc != 0:
                    raise RuntimeError(f"axon_start_nrt_profile rc={rc}")
                try:
                    yield
                finally:
                    n = lib.axon_stop_nrt_profile(str(output_dir).encode())
                    print(f"ntff profile: {n} file(s) -> {output_dir}",
                          file=sys.stderr)

            hook = _hook
    except OSError:
        pass
    mod = types.ModuleType("antenv.axon_hooks")
    mod.get_axon_ntff_profile_hook = lambda: hook
    mod.set_axon_ntff_profile_hook = lambda h: None
    sys.modules["antenv.axon_hooks"] = mod
    import antenv

    antenv.axon_hooks = mod


_install_ntff_hook_shim()

# ------------------------------------------------------------- constants ---
NC_N = 8
H = 1024
V = 50000
E = 602
S = 400
P = 128
VR = V // NC_N        # 6250 real vocab rows per core
MT = 49               # vocab m-tiles per core
VP = MT * P           # 6272 padded vocab rows per core
SP = 512              # padded S
ET_M = 4              # s-tiles (512/128)
EP_COLS = 640         # padded E (5*128)
APT = 5               # atten m-tiles

# packed-input free-dim offsets (bf16 elements per partition), 4 packs in
# arrival-priority order: p1a (GRU input weights) -> p1b (GRU hidden weights)
# -> p2a (attention score weights) -> p2b (everything else)
OFF_WIH = 0
OFF_X = OFF_WIH + 16 * 384
OFF_HV = OFF_X + 16
NB1A = OFF_HV + 16
OFF_WHH = 0
NB1B = 8 * 384
OFF_ATTN = 0
OFF_ET = OFF_ATTN + 1024
NB2A = OFF_ET + 8 * 512
OFF_EP = 0
OFF_CH = OFF_EP + 4 * 1024
OFF_CA = OFF_CH + 1024
OFF_PG = OFF_CA + 8 * 1024
OFF_WH = OFF_PG + 4 * 640
OFF_WS = OFF_WH + 16
OFF_WX = OFF_WS + 16
OFF_EMB = OFF_WX + 16
NB2B = OFF_EMB + 16
# f32 pack offsets
OFF_OUTB = 0
OFF_B01 = 56
OFF_BIHN = 64
OFF_BHHN = 72
OFF_CB = 80
OFF_HCOL = 88
OFF_CMASK = 96
OFF_SMASK = 104
OFF_CONSTS = 112
OFF_IDENT = 120
NF = 248

F32 = mybir.dt.float32
BF16 = mybir.dt.bfloat16
NPBF = ml_dtypes.bfloat16

LAST_RESULT = None    # BassKernelResults of the most recent run (for test.py)
TRACE = False         # set True (e.g. by test.py) to profile
DEBUG = False         # add per-stage debug outputs

_NC_CACHE = None



def _split_multi_waits(nc):
    """This walrus build accepts a single sync wait per instruction; hoist
    extra waits onto same-engine nops placed just before the instruction."""
    for f in nc.m.functions:
        for bb in f.blocks:
            out = []
            for ins in bb.instructions:
                si = ins.sync_info
                waits = list(si.on_wait) if si and si.on_wait else []
                if len(waits) > 1:
                    for w in waits[:-1]:
                        nop = mybir.InstNoOp(
                            name=nc.get_next_instruction_name(),
                            engine=ins.engine,
                            ins=[],
                            outs=[],
                            sync_info=mybir.SyncInfo(on_wait=[w], on_update=[]),
                        )
                        out.append(nop)
                    ins.sync_info = mybir.SyncInfo(
                        on_wait=[waits[-1]], on_update=si.on_update
                    )
                out.append(ins)
            bb.instructions = out


# ------------------------------------------------------------ device code ---
def _build_nc():
    nc = bass.Bass()

    def di(name, shape, dt=BF16):
        return nc.dram_tensor(name, shape, dt, kind="ExternalInput")

    # per-core inputs: one packed bf16 tensor, one packed f32 tensor, big W
    wo_t = di("wo_t", [H, VP])
    pack1a = di("pack1a", [P, NB1A])
    pack1b = di("pack1b", [P, NB1B])
    pack2a = di("pack2a", [P, NB2A])
    pack2b = di("pack2b", [P, NB2B])
    pack_f32 = di("pack_f32", [P, NF], F32)

    vocab_out = nc.dram_tensor("vocab_out", [P, MT], F32, kind="ExternalOutput")
    hnew_out = nc.dram_tensor("hnew_out", [P, 8], F32, kind="ExternalOutput")
    attnw_out = nc.dram_tensor("attnw_out", [P, 4], F32, kind="ExternalOutput")
    atten_out = nc.dram_tensor("atten_out", [P, APT], F32, kind="ExternalOutput")
    if DEBUG:
        dbg = {
            name: nc.dram_tensor(name, shape, F32, kind="ExternalOutput")
            for name, shape in [
                ("dbg_gi", [P, 3]), ("dbg_gh", [P, 3]), ("dbg_hnewl", [P, 1]),
                ("dbg_qp", [P, 8]), ("dbg_scl", [P, 4]), ("dbg_ar1", [P, 16]),
                ("dbg_aa", [P, 8]), ("dbg_ff", [P, 8]), ("dbg_pgen", [1, 1]),
            ]
        }

    with tile.TileContext(nc) as tc:
        with (
            tc.tile_pool(name="wp", bufs=1) as wp,
            tc.tile_pool(name="sp", bufs=1) as spool,
            tc.tile_pool(name="pp", bufs=1, space="PSUM") as pp,
            tc.tile_pool(name="bigp", bufs=1, space="PSUM") as bigp,
            tc.tile_pool(name="dram", bufs=1, space="DRAM") as dp,
        ):
            # ---- SBUF loads: 2 packed DMAs + 8 big-W chunk DMAs ----
            # (each dma_start costs ~1.1us of serial issue time on its queue,
            # so everything small rides in two packed transfers)
            pf32 = wp.tile([P, NF], F32, tag="pf32", name="pf32")
            nc.sync.dma_start(out=pf32[:], in_=pack_f32[:])
            pk1a = wp.tile([P, NB1A], BF16, tag="pk1a", name="pk1a")
            d1a = nc.sync.dma_start(out=pk1a[:], in_=pack1a[:])
            pk1b = wp.tile([P, NB1B], BF16, tag="pk1b", name="pk1b")
            d1b = nc.sync.dma_start(out=pk1b[:], in_=pack1b[:])
            pk2a = wp.tile([P, NB2A], BF16, tag="pk2a", name="pk2a")
            d2a = nc.sync.dma_start(out=pk2a[:], in_=pack2a[:])
            pk2b = wp.tile([P, NB2B], BF16, tag="pk2b", name="pk2b")
            d2b = nc.sync.dma_start(out=pk2b[:], in_=pack2b[:])
            add_dep_helper(d1b.ins, d1a.ins, reason="arrival priority")
            add_dep_helper(d2a.ins, d1b.ins, reason="arrival priority")
            add_dep_helper(d2b.ins, d2a.ins, reason="arrival priority")
            wo_sb = wp.tile([P, 8, VP], BF16, tag="wo", name="wo")
            for k in range(8):
                d_wo = nc.scalar.dma_start(
                    out=wo_sb[:, k, :], in_=wo_t[P * k : P * (k + 1), :]
                )
                add_dep_helper(d_wo.ins, d2a.ins, reason="packs get HBM first")

            wih = lambda k, m: pk1a[:, OFF_WIH + k * 384 + m * P :][:, :P]
            x_k = lambda k: pk1a[:, OFF_X + k : OFF_X + k + 1]
            h_k = lambda k: pk1a[:, OFF_HV + k : OFF_HV + k + 1]
            whh = lambda k, m: pk1b[:, OFF_WHH + k * 384 + m * P :][:, :P]
            attn_m = lambda m: pk2a[:, OFF_ATTN + m * P : OFF_ATTN + (m + 1) * P]
            et_km = lambda k, m: pk2a[:, OFF_ET + k * SP + m * P :][:, :P]
            ep_km = lambda k, m: pk2b[:, OFF_EP + k * H + m * P :][:, :P]
            ch_m = lambda m: pk2b[:, OFF_CH + m * P : OFF_CH + (m + 1) * P]
            ca_km = lambda k, m: pk2b[:, OFF_CA + k * H + m * P :][:, :P]
            pg_km = lambda k, m: pk2b[:, OFF_PG + k * EP_COLS + m * P :][:, :P]
            wh_k = lambda k: pk2b[:, OFF_WH + k : OFF_WH + k + 1]
            ws_sb = pk2b[:, OFF_WS : OFF_WS + 1]
            wx_k = lambda k: pk2b[:, OFF_WX + k : OFF_WX + k + 1]
            emb_k = lambda k: pk2b[:, OFF_EMB + k : OFF_EMB + k + 1]

            outb_sb = pf32[:, OFF_OUTB : OFF_OUTB + MT]
            b01_sb = pf32[:, OFF_B01 : OFF_B01 + 2]
            bihn_sb = pf32[:, OFF_BIHN : OFF_BIHN + 1]
            bhhn_sb = pf32[:, OFF_BHHN : OFF_BHHN + 1]
            cb_sb = pf32[:, OFF_CB : OFF_CB + 8]
            hcol_sb = pf32[:, OFF_HCOL : OFF_HCOL + 1]
            cmask_sb = pf32[:, OFF_CMASK : OFF_CMASK + 8]
            smask_sb = pf32[:, OFF_SMASK : OFF_SMASK + 4]
            consts_sb = pf32[0:1, OFF_CONSTS : OFF_CONSTS + 8]
            id_sb = pf32[:, OFF_IDENT : OFF_IDENT + P]

            onesc = spool.tile([P, 1], F32, tag="onesc")
            nc.vector.memset(onesc[:], 1.0)
            onesr = spool.tile([1, P], F32, tag="onesr")
            nc.vector.memset(onesr[:], 1.0)

            def psum(shape, tag="tiny", bufs=2, name="ps"):
                return pp.tile(list(shape), F32, tag=tag, bufs=bufs, name=name)

            def part_sum(vec_sb, k=P):
                """sum over partitions of [k,1] f32 -> [1,1] psum"""
                out = psum([1, 1])
                nc.tensor.matmul(out[:], onesc[:k, :], vec_sb, start=True, stop=True)
                return out

            def bcast(scalar_sb):
                """[1,1] sbuf f32 -> [128,1] sbuf f32"""
                pb = psum([P, 1])
                nc.tensor.matmul(pb[:], onesr[:], scalar_sb, start=True, stop=True)
                sb = spool.tile([P, 1], F32, tag="bc", name="bc")
                nc.vector.tensor_copy(sb[:], pb[:])
                return sb

            def to_sb(ps, shape, dt=F32, tag="cp"):
                sb = spool.tile(list(shape), dt, tag=tag, name=tag)
                nc.vector.tensor_copy(sb[:], ps)
                return sb

            # ---- stage 1: GRU slice (no comm) ----
            gi = psum([P, 3], tag="gates", name="gi")
            for k in range(16):
                for m in range(3):
                    nc.tensor.matmul(
                        gi[:, m : m + 1],
                        wih(k, m),
                        x_k(k),
                        start=(k == 0 and m == 0),
                        stop=(k == 15 and m == 2),
                    )
            gh = psum([P, 3], tag="gates", name="gh")
            for k in range(8):
                for m in range(3):
                    nc.tensor.matmul(
                        gh[:, m : m + 1],
                        whh(k, m),
                        h_k(k),
                        start=(k == 0 and m == 0),
                        stop=(k == 7 and m == 2),
                    )
            gi_sb = to_sb(gi[:], [P, 3], F32, tag="gisb")
            t01 = spool.tile([P, 2], F32, tag="t01")
            nc.vector.tensor_add(t01[:], gi_sb[:, 0:2], gh[:, 0:2])
            nc.vector.tensor_add(t01[:], t01[:], b01_sb)
            rz = spool.tile([P, 2], F32, tag="rz")
            nc.scalar.activation(rz[:], t01[:], mybir.ActivationFunctionType.Sigmoid)

            ghn = spool.tile([P, 1], F32, tag="ghn")
            nc.vector.tensor_add(ghn[:], gh[:, 2:3], bhhn_sb)
            tmp1 = spool.tile([P, 1], F32, tag="tmp1")
            nc.vector.tensor_mul(tmp1[:], rz[:, 0:1], ghn[:])
            npre = spool.tile([P, 1], F32, tag="npre")
            nc.vector.tensor_add(npre[:], gi_sb[:, 2:3], bihn_sb)
            nc.vector.tensor_add(npre[:], npre[:], tmp1[:])
            n_sb = spool.tile([P, 1], F32, tag="n")
            nc.scalar.activation(n_sb[:], npre[:], mybir.ActivationFunctionType.Tanh)
            warm = spool.tile([1, 1], F32, tag="warm")
            nc.scalar.activation(warm[:], consts_sb[0:1, 0:1],
                                 mybir.ActivationFunctionType.Exp)
            nc.scalar.activation(warm[:], consts_sb[0:1, 0:1],
                                 mybir.ActivationFunctionType.Ln)
            # h_new = n + z*(h - n)
            d_sb = spool.tile([P, 1], F32, tag="d")
            nc.vector.tensor_sub(d_sb[:], hcol_sb, n_sb[:])
            zt = spool.tile([P, 1], F32, tag="zt")
            nc.vector.tensor_mul(zt[:], rz[:, 1:2], d_sb[:])
            hnew = spool.tile([P, 1], F32, tag="hnew")
            nc.vector.tensor_add(hnew[:], n_sb[:], zt[:])
            hnew_bf = to_sb(hnew[:], [P, 1], BF16, tag="hnewbf")
            if DEBUG:
                nc.sync.dma_start(out=dbg["dbg_hnewl"][:], in_=hnew[:])
                nc.sync.dma_start(out=dbg["dbg_gi"][:], in_=gi_sb[:])
                gh_dbg = to_sb(gh[:], [P, 3], F32, tag="ghdbg")
                nc.sync.dma_start(out=dbg["dbg_gh"][:], in_=gh_dbg[:])

            # ---- stage 2: partial attention scores ----
            qp = psum([P, 8], tag="vec8", name="qp")
            for m in range(8):
                nc.tensor.matmul(
                    qp[:, m : m + 1],
                    attn_m(m),
                    hnew_bf[:],
                    start=(m == 0),
                    stop=(m == 7),
                )
            qp_bf = to_sb(qp[:], [P, 8], BF16, tag="qpbf")
            sc = psum([P, 4], tag="vec8", name="sc")
            for k in range(8):
                for m in range(4):
                    nc.tensor.matmul(
                        sc[:, m : m + 1],
                        et_km(k, m),
                        qp_bf[:, k : k + 1],
                        start=(k == 0 and m == 0),
                        stop=(k == 7 and m == 3),
                    )
            if DEBUG:
                qp_dbg = to_sb(qp[:], [P, 8], F32, tag="qpdbg")
                nc.sync.dma_start(out=dbg["dbg_qp"][:], in_=qp_dbg[:])
                sc_dbg = to_sb(sc[:], [P, 4], F32, tag="scdbg")
                nc.sync.dma_start(out=dbg["dbg_scl"][:], in_=sc_dbg[:])
            wsp = psum([1, 1], name="wsp")
            nc.tensor.matmul(wsp[:], ws_sb, hnew_bf[:], start=True, stop=True)
            # h-part of the combine FF, against the local h_new slice; rides
            # AllReduce #1 so the a-part (replicated) is all that remains after
            pfh = psum([P, 8], tag="vec8", name="pfh")
            for m in range(8):
                nc.tensor.matmul(
                    pfh[:, m : m + 1],
                    ch_m(m),
                    hnew_bf[:],
                    start=(m == 0),
                    stop=(m == 7),
                )

            # ---- AllReduce #1: scores + ws + h_new + ff-h-part ----
            ar1 = spool.tile([P, 24], F32, tag="ar1")
            nc.vector.memset(ar1[:], 0.0)
            nc.vector.tensor_copy(ar1[:, 0:4], sc[:])
            nc.vector.tensor_copy(ar1[0:1, 4:5], wsp[:])
            nc.vector.tensor_scalar_mul(ar1[:, 5:13], cmask_sb, hnew[:])
            nc.vector.tensor_copy(ar1[:, 16:24], pfh[:])
            ar1_in = dp.tile([P, 24], F32)
            ar1_out = dp.tile([P, 24], F32)
            nc.sync.dma_start(out=ar1_in[:], in_=ar1[:])
            nc.gpsimd.collective_compute(
                "AllReduce",
                mybir.AluOpType.add,
                replica_groups=[list(range(NC_N))],
                ins=[ar1_in.opt()],
                outs=[ar1_out.opt()],
            )
            # PE keep-warm during the AllReduce wait (dependency-free)
            warm_ps = psum([P, 1], tag="warmps", bufs=1, name="warmps")
            for w in range(60):
                nc.tensor.matmul(
                    warm_ps[:],
                    ch_m(w % 8),
                    hnew_bf[:],
                    start=(w == 0),
                    stop=(w == 59),
                )
            ag1 = spool.tile([P, 24], F32, tag="ag1")
            nc.sync.dma_start(out=ag1[:], in_=ar1_out[:])
            nc.sync.dma_start(out=hnew_out[:], in_=ag1[:, 5:13])
            if DEBUG:
                nc.sync.dma_start(out=dbg["dbg_ar1"][:], in_=ag1[:])

            # ---- stage 3: softmax(scores), attn_applied, p_gen (replicated) ----
            # scores span +-45 for this model scale -> exp safe in fp32
            # without max subtraction; softmax normalization is deferred so
            # the attn-applied matmuls start straight off the exp.
            scores = spool.tile([P, 4], F32, tag="scores")
            nc.vector.tensor_add(scores[:], ag1[:, 0:4], smask_sb)
            expsc = spool.tile([P, 4], F32, tag="expsc")
            rsum = spool.tile([P, 1], F32, tag="rsum")
            nc.scalar.activation(
                expsc[:], scores[:], mybir.ActivationFunctionType.Exp,
                accum_out=rsum[:],
            )
            aw_bf = to_sb(expsc[:], [P, 4], BF16, tag="awbf")  # unnormalized
            stot = to_sb(part_sum(rsum[:])[:], [1, 1], tag="stot")
            rinv = spool.tile([1, 1], F32, tag="rinv")
            nc.vector.reciprocal(rinv[:], stot[:])
            rinv_b = bcast(rinv[:])
            aw = spool.tile([P, 4], F32, tag="aw")
            nc.vector.tensor_scalar_mul(aw[:], expsc[:], rinv_b[:])
            nc.sync.dma_start(out=attnw_out[:], in_=aw[:])
            awn_bf = to_sb(aw[:], [P, 4], BF16, tag="awnbf")

            # attn_applied = attn_weights @ E  (full, replicated)
            aa = psum([P, 8], tag="vec8", name="aa")
            for k in range(ET_M):
                for m in range(8):
                    nc.tensor.matmul(
                        aa[:, m : m + 1],
                        ep_km(k, m),
                        aw_bf[:, k : k + 1],
                        start=(k == 0 and m == 0),
                        stop=(k == ET_M - 1 and m == 7),
                    )
            aa_bf = spool.tile([P, 8], BF16, tag="aabf")
            nc.vector.tensor_scalar_mul(aa_bf[:], aa[:], rinv_b[:])
            # a-part of the combine FF against full (replicated) weights
            pfa = psum([P, 8], tag="vec8", name="pfa")
            for k in range(8):
                for m in range(8):
                    nc.tensor.matmul(
                        pfa[:, m : m + 1],
                        ca_km(k, m),
                        aa_bf[:, k : k + 1],
                        start=(k == 0 and m == 0),
                        stop=(k == 7 and m == 7),
                    )
            pff = spool.tile([P, 8], F32, tag="pff")
            nc.vector.tensor_add(pff[:], ag1[:, 16:24], pfa[:])
            nc.vector.tensor_add(pff[:], pff[:], cb_sb)
            ff_bf = spool.tile([P, 8], BF16, tag="ffbf")
            nc.scalar.activation(ff_bf[:], pff[:], mybir.ActivationFunctionType.Relu)
            if DEBUG:
                ff = spool.tile([P, 8], F32, tag="ff")
                nc.scalar.activation(ff[:], pff[:],
                                     mybir.ActivationFunctionType.Relu)
                nc.sync.dma_start(out=dbg["dbg_ff"][:], in_=ff[:])

            # ---- big matvec: logits shard [128, 49] ----
            big = bigp.tile([P, MT], F32, tag="big")
            for k in range(8):
                for j in range(MT):
                    nc.tensor.matmul(
                        big[:, j : j + 1],
                        wo_sb[:, k, j * P : (j + 1) * P],
                        ff_bf[:, k : k + 1],
                        start=(k == 0 and j == 0),
                        stop=(k == 7 and j == MT - 1),
                    )
            logits = spool.tile([P, MT], F32, tag="logits")
            nc.vector.tensor_add(logits[:], big[:], outb_sb)
            expv = spool.tile([P, MT], F32, tag="expv")
            esum = spool.tile([P, 1], F32, tag="esum")
            nc.scalar.activation(
                expv[:], logits[:], mybir.ActivationFunctionType.Exp,
                accum_out=esum[:],
            )
            se = to_sb(part_sum(esum[:])[:], [1, 1], tag="se")

            # ---- AllGather #3: per-core sumexp ----
            ag3 = spool.tile([1, 8], F32, tag="ag3")
            nc.vector.memset(ag3[:], 0.0)
            nc.vector.tensor_copy(ag3[0:1, 0:1], se[:])
            ag3_in = dp.tile([1, 8], F32)
            # AG concat is on the partition axis, but DRAM is linear: a
            # [1, 64] view of the same bytes gives the 8 rank rows flat.
            ag3_out = dp.tile([1, 64], F32)
            nc.sync.dma_start(out=ag3_in[:], in_=ag3[:])
            nc.gpsimd.collective_compute(
                "AllGather",
                mybir.AluOpType.bypass,
                replica_groups=[list(range(NC_N))],
                ins=[ag3_in.opt()],
                outs=[ag3_out.opt()],
            )
            # p_gen + atten_p: independent of the vocab chain; scheduled here
            # so they fill the AllGather wait window
            pgp = psum([1, 1], name="pgp")
            for k in range(8):
                nc.tensor.matmul(
                    pgp[:], wh_k(k), aa_bf[:, k : k + 1],
                    start=(k == 0), stop=False,
                )
            for k in range(8):
                nc.tensor.matmul(
                    pgp[:], wx_k(k), emb_k(k),
                    start=False, stop=(k == 7),
                )
            p1 = spool.tile([1, 1], F32, tag="p1")
            nc.vector.tensor_add(p1[:], pgp[:], ag1[0:1, 4:5])
            pgen = spool.tile([1, 1], F32, tag="pgen")
            nc.scalar.activation(
                pgen[:], p1[:], mybir.ActivationFunctionType.Sigmoid,
                bias=consts_sb[0:1, 1:2],
            )
            ln_pg = spool.tile([1, 1], F32, tag="lnpg")
            nc.scalar.activation(ln_pg[:], pgen[:], mybir.ActivationFunctionType.Ln)
            om = spool.tile([1, 1], F32, tag="om")
            nc.vector.tensor_sub(om[:], consts_sb[0:1, 0:1], pgen[:])
            ln_om = spool.tile([1, 1], F32, tag="lnom")
            nc.scalar.activation(ln_om[:], om[:], mybir.ActivationFunctionType.Ln)
            if DEBUG:
                nc.sync.dma_start(out=dbg["dbg_pgen"][:], in_=pgen[:])

            app = psum([P, APT], tag="vec8", name="app")
            for k in range(ET_M):
                for m in range(APT):
                    nc.tensor.matmul(
                        app[:, m : m + 1],
                        pg_km(k, m),
                        awn_bf[:, k : k + 1],
                        start=(k == 0 and m == 0),
                        stop=(k == ET_M - 1 and m == APT - 1),
                    )
            ln_ap = spool.tile([P, APT], F32, tag="lnap")
            nc.scalar.activation(ln_ap[:], app[:], mybir.ActivationFunctionType.Ln)
            lnom_b = bcast(ln_om[:])
            apf = spool.tile([P, APT], F32, tag="apf")
            nc.vector.tensor_scalar_add(apf[:], ln_ap[:], lnom_b[:])
            nc.sync.dma_start(out=atten_out[:], in_=apf[:])

            agd = spool.tile([1, 64], F32, tag="agd")
            nc.sync.dma_start(out=agd[:], in_=ag3_out[:])
            tot = spool.tile([1, 1], F32, tag="tot")
            nc.vector.reduce_sum(out=tot[:], in_=agd[:], axis=mybir.AxisListType.X)
            lnz = spool.tile([1, 1], F32, tag="lnz")
            nc.scalar.activation(lnz[:], tot[:], mybir.ActivationFunctionType.Ln)
            corr = spool.tile([1, 1], F32, tag="corr")
            nc.vector.tensor_sub(corr[:], lnz[:], ln_pg[:])
            corr_b = bcast(corr[:])
            final = spool.tile([P, MT], F32, tag="final")
            nc.vector.tensor_scalar_sub(final[:], logits[:], corr_b[:])
            nc.sync.dma_start(out=vocab_out[:], in_=final[:])

    _split_multi_waits(nc)
    return nc


# -------------------------------------------------------------- host side ---
def _colmajor(v, ncol):
    return np.ascontiguousarray(v.reshape(ncol, P).T)


def _prep_inputs(inputs):
    f32 = np.float32
    idx = int(np.asarray(inputs["input_idx"]).ravel()[0])
    emb = np.asarray(inputs["emb"], f32)
    embedded = emb[idx]
    trigger = np.asarray(inputs["trigger"], f32)
    x = np.concatenate([embedded, trigger])
    h = np.asarray(inputs["hidden"], f32)[0, 0]
    enc = np.asarray(inputs["encoder_outputs"], f32)
    pg_mat = np.asarray(inputs["pg_mat"], f32)
    attn_W = np.asarray(inputs["attn_W"], f32)
    comb_W = np.asarray(inputs["comb_W"], f32)
    comb_b = np.asarray(inputs["comb_b"], f32)
    W_ih = np.asarray(inputs["W_ih"], f32)
    W_hh = np.asarray(inputs["W_hh"], f32)
    b_ih = np.asarray(inputs["b_ih"], f32)
    b_hh = np.asarray(inputs["b_hh"], f32)
    out_W = np.asarray(inputs["out_W"], f32)
    out_b = np.asarray(inputs["out_b"], f32)
    wh_W = np.asarray(inputs["wh_W"], f32)[0]
    ws_W = np.asarray(inputs["ws_W"], f32)[0]
    wx_W = np.asarray(inputs["wx_W"], f32)[0]
    wx_b = np.asarray(inputs["wx_b"], f32)[0]

    et = np.zeros((H, SP), f32)
    et[:, :S] = enc.T
    ep = np.zeros((SP, H), f32)
    ep[:S] = enc
    pgp = np.zeros((SP, EP_COLS), f32)
    pgp[:S, :E] = pg_mat
    pgp[:S, E:] = 1.0
    sm_flat = np.zeros(SP, f32)
    sm_flat[S:] = -1e30

    def chunked(a, k, m):
        """[k*128, m] row-major -> [128, k*m]  ("(k p) m -> p (k m)")"""
        return a.reshape(k, P, m).transpose(1, 0, 2).reshape(P, k * m)

    def pad16(col):
        out = np.zeros((P, 16), f32)
        out[:, : col.shape[1]] = col
        return out

    # shared bf16 pack pieces (order must match OFF_* in the device code)
    et_pk = chunked(et, 8, SP)
    ep_pk = chunked(ep, ET_M, H)
    pg_pk = chunked(pgp, ET_M, EP_COLS)
    caT_pk = chunked(np.ascontiguousarray(comb_W[:, H:].T), 8, H)
    wh_pk = pad16(_colmajor(wh_W, 8))
    wx_pk = pad16(_colmajor(wx_W, 8))
    x_pk = pad16(_colmajor(x, 16))
    h_pk = pad16(_colmajor(h, 8))
    emb_pk = pad16(_colmajor(embedded, 8))

    ident = np.eye(P, dtype=f32)
    consts_col = np.zeros((P, 8), f32)
    consts_col[0, 0] = 1.0
    consts_col[0, 1] = wx_b
    smask_col = _colmajor(sm_flat, 4)
    cb_col = _colmajor(comb_b, 8)

    in_maps = []
    for c in range(NC_N):
        s = slice(P * c, P * (c + 1))
        rows = np.r_[P * c : P * (c + 1), H + P * c : H + P * (c + 1),
                     2 * H + P * c : 2 * H + P * (c + 1)]
        wsh = np.zeros((VP, H), f32)
        wsh[:VR] = out_W[VR * c : VR * (c + 1)]
        ob = np.full(VP, -40.0, f32)
        ob[:VR] = out_b[VR * c : VR * (c + 1)]
        cmask = np.zeros((P, 8), f32)
        cmask[:, c] = 1.0

        p1a = np.zeros((P, NB1A), f32)
        p1a[:, OFF_WIH : OFF_WIH + 16 * 384] = chunked(
            np.ascontiguousarray(W_ih[rows].T), 16, 384)
        p1a[:, OFF_X : OFF_X + 16] = x_pk
        p1a[:, OFF_HV : OFF_HV + 16] = h_pk

        p1b = np.zeros((P, NB1B), f32)
        p1b[:, OFF_WHH : OFF_WHH + 8 * 384] = chunked(
            np.ascontiguousarray(W_hh[rows].T), 8, 384)

        p2a = np.zeros((P, NB2A), f32)
        p2a[:, OFF_ATTN : OFF_ATTN + H] = attn_W[:, s].T
        p2a[:, OFF_ET : OFF_ET + 8 * SP] = et_pk

        p2b = np.zeros((P, NB2B), f32)
        p2b[:, OFF_EP : OFF_EP + ET_M * H] = ep_pk
        p2b[:, OFF_CH : OFF_CH + H] = comb_W[:, s].T
        p2b[:, OFF_CA : OFF_CA + 8 * H] = caT_pk
        p2b[:, OFF_PG : OFF_PG + ET_M * EP_COLS] = pg_pk
        p2b[:, OFF_WH : OFF_WH + 16] = wh_pk
        p2b[:, OFF_WS] = ws_W[s]
        p2b[:, OFF_WX : OFF_WX + 16] = wx_pk
        p2b[:, OFF_EMB : OFF_EMB + 16] = emb_pk

        pack_f32 = np.zeros((P, NF), f32)
        pack_f32[:, OFF_OUTB : OFF_OUTB + MT] = _colmajor(ob, MT)
        pack_f32[:, OFF_B01] = b_ih[s] + b_hh[s]
        pack_f32[:, OFF_B01 + 1] = (b_ih[H + P * c : H + P * (c + 1)]
                                    + b_hh[H + P * c : H + P * (c + 1)])
        pack_f32[:, OFF_BIHN] = b_ih[2 * H + P * c : 2 * H + P * (c + 1)]
        pack_f32[:, OFF_BHHN] = b_hh[2 * H + P * c : 2 * H + P * (c + 1)]
        pack_f32[:, OFF_CB : OFF_CB + 8] = cb_col
        pack_f32[:, OFF_HCOL] = h[s]
        pack_f32[:, OFF_CMASK : OFF_CMASK + 8] = cmask
        pack_f32[:, OFF_SMASK : OFF_SMASK + 4] = smask_col
        pack_f32[:, OFF_CONSTS : OFF_CONSTS + 8] = consts_col
        pack_f32[:, OFF_IDENT : OFF_IDENT + P] = ident

        m = {
            "wo_t": np.ascontiguousarray(wsh.T).astype(NPBF),
            "pack1a": p1a.astype(NPBF),
            "pack1b": p1b.astype(NPBF),
            "pack2a": p2a.astype(NPBF),
            "pack2b": p2b.astype(NPBF),
            "pack_f32": pack_f32,
        }
        in_maps.append(m)
    return in_maps


def kernel(**inputs):
    global _NC_CACHE, LAST_RESULT
    in_maps = _prep_inputs(inputs)
    if _NC_CACHE is None:
        _NC_CACHE = _build_nc()
    res = run_bass_kernel_spmd(
        _NC_CACHE, in_maps, list(range(NC_N)), trace=TRACE
    )
    LAST_RESULT = res

    vocab = np.concatenate(
        [res.results[c]["vocab_out"].T.reshape(-1)[:VR] for c in range(NC_N)]
    )
    atten = res.results[0]["atten_out"].T.reshape(-1)[:E]
    output = np.concatenate([vocab, atten])[None, :].astype(np.float32)
    h_new = res.results[0]["hnew_out"].T.reshape(-1)[None, None, :].astype(np.float32)
    attn_weights = (
        res.results[0]["attnw_out"].T.reshape(-1)[:S][None, :].astype(np.float32)
    )
    return output, h_new, attn_weights
